# revision 5
# baseline (speedup 1.0000x reference)
"""Trainium2 Bass kernel for LlamaMultiheadLatentAttention.

Contract: kernel(**inputs) takes FULL fp32 inputs (as produced by
reference.setup_inputs) and returns the FULL fp32 output [2, 1024, 4096].

Sharding (8 cores, no collectives): core c handles batch b = c//4 and
head-group g = c%4 (8 query heads, 2 kv heads, 8 latent heads). q/k/v and
latent projections are column-sharded per head-group; o_proj/latent_o_proj
are row-sharded, so each core emits a partial output sum and the host adds
the 4 partials per batch (the "all-reduce" of the output happens at unshard
time on the host).

v3 scheduling changes (on top of v2):
  - B2 DMAs and matmul loop are kt-granular/kt-major: first matmul fires
    after one 384KB chunk instead of 1.5MB.
  - the first two attention units' scores are hoisted into the B1 pool
    scope, with the ps_s PSUM pool allocated BEFORE ps_b1 so the score
    matmuls land in banks that don't wait for the last rope's PSUM reads
    (kills the ~4us B1->attention PE stall).
  - ib=0 softmax denominators use the DVE presum + single f32r ones-matmul
    path (like ib=1) instead of 4 PE matmuls: -768 PE cycles/unit.
  - the last output-projection chunk is split in two 256-col halves so the
    final copy+DMA tail is shorter.
"""

import numpy as np
import ml_dtypes

import concourse.bass as bass
import concourse.mybir as mybir
import concourse.tile as tile
from concourse import bacc
from concourse.bass_utils import run_bass_kernel_spmd

BF16 = ml_dtypes.bfloat16

B, S, D = 2, 1024, 4096
H, KVH, HD = 32, 8, 128
GROUPS = H // KVH
LAT, LH = 1024, 32
THETA = 10000.0
SCALE = 1.0 / float(np.sqrt(HD))

NCORES = 8
TP = 4                 # head-group shards
HL = H // TP           # 8 local q heads
KVL = KVH // TP        # 2 local kv heads
LHL = LH // TP         # 8 local latent heads

f32 = mybir.dt.float32
bf16 = mybir.dt.bfloat16

D_T = D // 128         # 32 k-tiles over model dim
S_T = S // 128         # 8 token tiles of 128
IB = 2                 # token blocks of 512
NB = D // 512          # 8 output column blocks


def _build_program():
    nc = bacc.Bacc("TRN2", target_bir_lowering=False, debug=False)

    xt_d = nc.dram_tensor("xt", [128, D_T, S], bf16, kind="ExternalInput")
    wq_d = nc.dram_tensor("wq", [HL, 128, D_T, 128], bf16, kind="ExternalInput")
    wk_d = nc.dram_tensor("wk", [KVL, 128, D_T, 128], bf16, kind="ExternalInput")
    wvc_d = nc.dram_tensor("wvc", [128, D_T, 512], bf16, kind="ExternalInput")
    w2_d = nc.dram_tensor("w2", [LHL, 128, D_T, 128], bf16, kind="ExternalInput")
    wlvb_d = nc.dram_tensor("wlvb", [128, D_T, 512], bf16, kind="ExternalInput")
    wlvc_d = nc.dram_tensor("wlvc", [128, D_T, 256], bf16, kind="ExternalInput")
    wo_d = nc.dram_tensor("wo", [NB, 128, HL, 512], bf16, kind="ExternalInput")
    wlo_d = nc.dram_tensor("wlo", [NB, 128, LHL, 512], bf16, kind="ExternalInput")
    cos_d = nc.dram_tensor("cosT", [HD, S], f32, kind="ExternalInput")
    sin_d = nc.dram_tensor("sinTs", [HD, S], f32, kind="ExternalInput")
    mask_d = nc.dram_tensor("maskD", [128, 128], bf16, kind="ExternalInput")
    out_d = nc.dram_tensor("out", [S, D], f32, kind="ExternalOutput")

    out_ap = out_d.ap().rearrange("(tt p) d -> p tt d", p=128)

    with tile.TileContext(nc) as tc:
        with tc.tile_pool(name="const", bufs=1) as constp, \
             tc.tile_pool(name="acts", bufs=1) as acts, \
             tc.tile_pool(name="pp", bufs=16) as pp:
            # ps_s opens only after B2's 8-bank PSUM pool closes (3+8 > 8
            # banks), but must outlive the xt scope: managed manually.
            pss_cm = tc.tile_pool(name="ps_s", bufs=3, space="PSUM")
            pss_ = None

            # persistent activations (bf16); q/k/lk tiles allocated at B1
            v_sb = acts.tile([128, S_T, KVL * HD], bf16, tag="v")
            lv_sb = acts.tile([128, S_T, LHL * HD], bf16, tag="lv")
            maskD = acts.tile([128, 128], bf16, tag="maskD")
            ones = acts.tile([128, 128], bf16, tag="ones")
            ones_f = acts.tile([128, 128], f32, tag="ones_f")
            ones32 = acts.tile([128, 128], mybir.dt.float32r, tag="ones32")
            nc.sync.dma_start(maskD[:], mask_d.ap())
            nc.vector.memset(ones[:], 1.0)
            nc.vector.memset(ones_f[:], 1.0)
            nc.vector.tensor_copy(ones32[:], ones_f[:])

            # late-bound attention state (activation tiles created in later
            # scopes; emit closures look them up at call time)
            st = {}
            _uid = [0]

            def uname(pfx):
                _uid[0] += 1
                return f"{pfx}_{_uid[0]}"

            def vh_parts(vh):
                if vh < HL:
                    h = vh
                    return (h, st['kT'][:, h // GROUPS, :], 'attnT',
                            lambda jb: v_sb[:, jb, bass.ts(h // GROUPS, HD)])
                h = vh - HL
                return (h, st['lkT'][:, h, :], 'latT',
                        lambda jb: lv_sb[:, jb, bass.ts(h, HD)])

            def off_of(jb, ib):
                return max(jb - 4 * ib, 0) * 128

            def emit_scores(u, half):
                # scores + exp + mask for jbs of one half
                vh, ib, pts = u
                h, ksrc, _, _ = vh_parts(vh)
                njb = 4 * (ib + 1)
                lo = 0 if half == 0 else njb // 2
                hi = njb // 2 if half == 0 else njb
                for jb in range(lo, hi):
                    off = off_of(jb, ib)
                    ps_s = pss_.tile([128, 512], f32, tag="ps_s",
                                     name=uname("ps_s"))
                    nc.tensor.matmul(
                        ps_s[:, off:512],
                        ksrc[:, bass.ts(jb, 128)],
                        st['qT'][:, h, bass.ds(ib * 512 + off, 512 - off)],
                        start=True, stop=True)
                    pt = pp.tile([128, 512], bf16, tag="pt", name=uname("pt"))
                    nc.scalar.activation(
                        pt[:, off:512], ps_s[:, off:512],
                        mybir.ActivationFunctionType.Exp,
                        scale=SCALE)
                    if jb >= 4 * ib:
                        nc.gpsimd.tensor_mul(
                            pt[:, off:off + 128],
                            pt[:, off:off + 128], maskD[:])
                    pts.append(pt)

            with tc.tile_pool(name="xt", bufs=1) as xtp:
                xt = xtp.tile([128, D_T, S], bf16, tag="xt")

                # preload zone: B1's first weight tile + rope tables live in
                # memory that is never recycled from B2's streaming pools, so
                # their DMAs run during B2 compute instead of after it.
                wpre = xtp.tile([128, D_T, 128], bf16, tag="wpre")
                cosT = xtp.tile([HD, S], f32, tag="cosT")
                sinTs = xtp.tile([HD, S], f32, tag="sinTs")

                # ---- phase B2: token-major projections v, lv ----
                # three column passes (A = [wlv 0:256 | wv], B = wlv 256:768,
                # C = wlv 768:1024), kt-major so the PE consumes input
                # chunks in DMA arrival order; one PSUM bank per token tile.
                # B2 weights stream through 8-kt chunk tiles (4 bufs per
                # tag) so the pool costs 48KB instead of 96KB. DMA order:
                # xt+wA interleaved kt-granular (first matmul after ~384KB),
                # then wB, then the B1 preloads, then wC last (wC aliasing
                # can stall its queue; nothing important sits behind it).
                CH = 8
                with tc.tile_pool(name="wb2", bufs=4) as wb2p, \
                     tc.tile_pool(name="ps_b2", bufs=8, space="PSUM") as psb2:
                    wAc = [wb2p.tile([128, CH, 512], bf16, tag="w512",
                                     name=f"wA{i}") for i in range(4)]
                    wBc = [wb2p.tile([128, CH, 512], bf16, tag="w512",
                                     name=f"wB{i}") for i in range(4)]
                    wCc = [wb2p.tile([128, CH, 256], bf16, tag="w256",
                                     name=f"wC{i}") for i in range(4)]
                    for kt in range(D_T):
                        nc.sync.dma_start(
                            xt[:, bass.ts(kt, 1), :],
                            xt_d.ap()[:, bass.ts(kt, 1), :])
                        nc.sync.dma_start(
                            wAc[kt // CH][:, bass.ts(kt % CH, 1), :],
                            wvc_d.ap()[:, bass.ts(kt, 1), :])
                    for c in range(4):
                        nc.sync.dma_start(
                            wBc[c][:],
                            wlvb_d.ap()[:, bass.ts(c, CH), :])
                    nc.sync.dma_start(wpre[:], wq_d.ap()[0])
                    nc.sync.dma_start(cosT[:], cos_d.ap())
                    nc.sync.dma_start(sinTs[:], sin_d.ap())
                    for c in range(4):
                        nc.sync.dma_start(
                            wCc[c][:],
                            wlvc_d.ap()[:, bass.ts(c, CH), :])

                    for p, (wcs, ncols) in enumerate(
                            ((wAc, 512), (wBc, 512), (wCc, 256))):
                        pss_b2 = [psb2.tile([128, 512], f32, tag="ps_b2",
                                            name=f"psb2_{p}_{tt}")
                                  for tt in range(S_T)]
                        for kt in range(D_T):
                            wt = wcs[kt // CH]
                            for tt in range(S_T):
                                nc.tensor.matmul(
                                    pss_b2[tt][:, 0:ncols],
                                    xt[:, kt, bass.ts(tt, 128)],
                                    wt[:, kt % CH, 0:ncols],
                                    start=(kt == 0), stop=(kt == D_T - 1))
                                if kt == D_T - 1:
                                    ps = pss_b2[tt]
                                    if p == 0:
                                        nc.any.tensor_copy(
                                            lv_sb[:, tt, 0:256], ps[:, 0:256])
                                        nc.any.tensor_copy(
                                            v_sb[:, tt, :], ps[:, 256:512])
                                    elif p == 1:
                                        nc.any.tensor_copy(
                                            lv_sb[:, tt, 256:768], ps[:])
                                    else:
                                        nc.any.tensor_copy(
                                            lv_sb[:, tt, 768:1024],
                                            ps[:, 0:256])
                pss_ = pss_cm.__enter__()

                # ---- phase B1: feature-major projections q, k, lk (+rope) --
                qT = acts.tile([128, HL, S], bf16, tag="qT")
                kT = acts.tile([128, KVL, S], bf16, tag="kT")
                lkT = acts.tile([128, LHL, S], bf16, tag="lkT")
                st['qT'], st['kT'], st['lkT'] = qT, kT, lkT
                with tc.tile_pool(name="wstr", bufs=3) as wstr, \
                     tc.tile_pool(name="rope", bufs=4) as ropep, \
                     tc.tile_pool(name="ps_b1", bufs=4, space="PSUM") as psb1:

                    def rope_to(dst, ps, ib):
                        sl = bass.ts(ib, 512)
                        rt = ropep.tile([128, 512], f32, tag="rt",
                                        name=uname("rt"))
                        qc = ropep.tile([128, 512], f32, tag="qc",
                                        name=uname("qc"))
                        nc.vector.tensor_tensor(
                            rt[0:64, :], ps[64:128, :], sinTs[0:64, sl],
                            mybir.AluOpType.mult)
                        nc.vector.tensor_tensor(
                            rt[64:128, :], ps[0:64, :], sinTs[64:128, sl],
                            mybir.AluOpType.mult)
                        nc.vector.tensor_tensor(
                            qc[:], ps[:], cosT[:, sl], mybir.AluOpType.mult)
                        nc.vector.tensor_add(dst, qc[:], rt[:])

                    def proj_fm(w_dram, n_tiles, dst, pre=None):
                        for nt in range(n_tiles):
                            if nt == 0 and pre is not None:
                                wt = pre
                            else:
                                wt = wstr.tile([128, D_T, 128], bf16,
                                               tag="w_fm", name=uname("w_fm"))
                                nc.sync.dma_start(wt[:], w_dram.ap()[nt])
                            ps = [psb1.tile([128, 512], f32, tag="ps_b1",
                                            name=uname("ps_b1"))
                                  for ib in range(IB)]
                            for kt in range(D_T):
                                for ib in range(IB):
                                    nc.tensor.matmul(
                                        ps[ib][:], wt[:, kt, :],
                                        xt[:, kt, bass.ts(ib, 512)],
                                        start=(kt == 0), stop=(kt == D_T - 1))
                            for ib in range(IB):
                                rope_to(dst[:, nt, bass.ts(ib, 512)],
                                        ps[ib][:], ib)

                    proj_fm(wq_d, HL, qT, pre=wpre)
                    proj_fm(wk_d, KVL, kT)
                    proj_fm(w2_d, LHL, lkT)

                    # hoist the first two attention units' scores: their
                    # PSUM banks (ps_s, allocated before ps_b1) are free, so
                    # the PE rolls straight from the last projection matmul
                    # into attention while the tail rope drains on DVE.
                    u0 = [0, 0, []]
                    emit_scores(u0, 0)
                    emit_scores(u0, 1)
                    u1 = [1, 0, []]
                    emit_scores(u1, 0)
                    emit_scores(u1, 1)

            # ---- phase C+D: attention with output-projection weave ----
            with tc.tile_pool(name="attnlat", bufs=1) as alp:
                attnT = alp.tile([128, HL, S], bf16, tag="attnT")
                latT = alp.tile([128, LHL, S], bf16, tag="latT")
                st['attnT'], st['latT'] = attnT, latT

                with tc.tile_pool(name="dn", bufs=3) as dn, \
                     tc.tile_pool(name="wop", bufs=4) as wop, \
                     tc.tile_pool(name="ost", bufs=6) as ost:
                    wo_seq = [0]

                    def wo_dma(nb):
                        sq = wo_seq[0]
                        wo_seq[0] += 1
                        w = wop.tile([128, HL, 512], bf16, tag="wo",
                                     name=f"wo_{sq}")
                        wl = wop.tile([128, LHL, 512], bf16, tag="wlo",
                                      name=f"wlo_{sq}")
                        nc.sync.dma_start(w[:], wo_d.ap()[nb])
                        nc.sync.dma_start(wl[:], wlo_d.ap()[nb])
                        return w, wl

                    def d_chunk(psf_pool, wpair, nb, tt, c0=0, cw=512):
                        w, wl = wpair
                        psf = psf_pool.tile([128, 512], f32, tag="psf",
                                            name=uname("psf"))
                        for h in range(HL):
                            nc.tensor.matmul(
                                psf[:, 0:cw],
                                attnT[:, h, bass.ts(tt, 128)],
                                w[:, h, c0:c0 + cw],
                                start=(h == 0), stop=False)
                        for h in range(LHL):
                            nc.tensor.matmul(
                                psf[:, 0:cw],
                                latT[:, h, bass.ts(tt, 128)],
                                wl[:, h, c0:c0 + cw],
                                start=False, stop=(h == LHL - 1))
                        ot = ost.tile([128, 512], f32, tag="ot",
                                      name=uname("ot"))
                        nc.vector.tensor_copy(ot[:, 0:cw], psf[:, 0:cw])
                        nc.sync.dma_start(
                            out_ap[:, tt, bass.ds(nb * 512 + c0, cw)],
                            ot[:, 0:cw])

                    with tc.tile_pool(name="ps_f", bufs=2,
                                      space="PSUM") as psf_, \
                         tc.tile_pool(name="ps_o", bufs=2,
                                      space="PSUM") as pso_, \
                         tc.tile_pool(name="ps_d", bufs=1,
                                      space="PSUM") as psd_:

                        def emit_pv(u):
                            vh, ib, pts = u
                            _, _, _, vsl = vh_parts(vh)
                            njb = 4 * (ib + 1)
                            u.append(pso_.tile([128, 512], f32, tag="ps_o",
                                               name=uname("ps_o")))
                            ps_o = u[3]
                            for jb in range(njb):
                                off = off_of(jb, ib)
                                nc.tensor.matmul(
                                    ps_o[:, off:512], vsl(jb),
                                    pts[jb][:, off:512],
                                    start=(jb == 0), stop=(jb == njb - 1))

                        def emit_den_norm(u):
                            vh, ib, pts, ps_o = u
                            h, _, dstk, _ = vh_parts(vh)
                            dst = st[dstk]
                            njb = 4 * (ib + 1)
                            ps_d = psd_.tile([128, 512], f32, tag="ps_d",
                                             name=uname("ps_d"))
                            # pre-sum the P tiles on DVE (f32r), then one
                            # f32r ones-matmul does the partition reduce
                            acc = dn.tile([128, 512],
                                          mybir.dt.float32r, tag="acc",
                                          name=uname("acc"))
                            if ib == 1:
                                nc.vector.tensor_add(acc[:], pts[0][:],
                                                     pts[1][:])
                                jb0 = 2
                            else:
                                nc.vector.tensor_copy(acc[:], pts[0][:])
                                jb0 = 1
                            for jb in range(jb0, njb):
                                off = off_of(jb, ib)
                                nc.vector.tensor_add(
                                    acc[:, off:512], acc[:, off:512],
                                    pts[jb][:, off:512])
                            nc.tensor.matmul(
                                ps_d[:], ones32[:], acc[:],
                                start=True, stop=True)
                            rec = dn.tile([128, 512], f32, tag="rec",
                                          name=uname("rec"))
                            nc.vector.reciprocal_approx_fast(rec[:], ps_d[:])
                            nc.vector.tensor_tensor(
                                dst[:, h, bass.ts(ib, 512)], ps_o[:], rec[:],
                                mybir.AluOpType.mult)

                        # --- pass ib=0: software-pipelined, two units of lag
                        # (units 0 and 1 were hoisted into the B1 scope)
                        wpairs = {0: wo_dma(0), 1: wo_dma(1)}
                        prevq = [u0, u1]
                        for vh in range(2, HL + LHL):
                            u = [vh, 0, []]
                            emit_scores(u, 0)
                            emit_scores(u, 1)
                            pu = prevq.pop(0)
                            emit_pv(pu)
                            emit_den_norm(pu)
                            prevq.append(u)
                        # drain to one unit of lag before the ib=1 pass
                        pu = prevq.pop(0)
                        emit_pv(pu)
                        emit_den_norm(pu)
                        prev = prevq.pop(0)

                        # --- pass ib=1 with output-projection chunks woven
                        chunks = [(nb, tt) for nb in range(NB)
                                  for tt in range(S_T // 2)]
                        ci = 0
                        for vh in range(HL + LHL):
                            u = [vh, 1, []]
                            emit_scores(u, 0)
                            emit_pv(prev)
                            emit_scores(u, 1)
                            emit_den_norm(prev)
                            prev = u
                            for _ in range(2):
                                if ci < len(chunks):
                                    nb, tt = chunks[ci]
                                    for ahead in (1, 2):
                                        nba = nb + ahead
                                        if (nba < NB and nba not in wpairs
                                                and tt == 2 * ahead - 2):
                                            wpairs[nba] = wo_dma(nba)
                                    d_chunk(psf_, wpairs[nb], nb, tt)
                                    wpairs.pop(nb - 1, None)
                                    ci += 1
                        emit_pv(prev)
                        emit_den_norm(prev)

                    # --- output-projection tail: token tiles 4..7
                    with tc.tile_pool(name="ps_f2", bufs=4,
                                      space="PSUM") as psf2_:
                        tpair = {0: wo_dma(0), 1: wo_dma(1)}
                        for nb in range(NB):
                            for tt in range(S_T // 2, S_T):
                                if tt == S_T // 2 and nb + 2 < NB:
                                    tpair[nb + 2] = wo_dma(nb + 2)
                                if nb == NB - 1 and tt == S_T - 1:
                                    # split the last chunk: shorter end tail
                                    d_chunk(psf2_, tpair[nb], nb, tt,
                                            c0=0, cw=256)
                                    d_chunk(psf2_, tpair[nb], nb, tt,
                                            c0=256, cw=256)
                                else:
                                    d_chunk(psf2_, tpair[nb], nb, tt)
                            tpair.pop(nb, None)

            pss_cm.__exit__(None, None, None)

    nc.compile()
    return nc


_NC = None


def _get_program():
    global _NC
    if _NC is None:
        _NC = _build_program()
    return _NC


def _rope_tables():
    inv_freq = 1.0 / (THETA ** (np.arange(0, HD, 2, dtype=np.float32) / HD))
    t = np.arange(S, dtype=np.float32)
    freqs = np.outer(t, inv_freq)                       # [S, 64]
    emb = np.concatenate([freqs, freqs], axis=-1)       # [S, HD]
    cosT = np.cos(emb).T.astype(np.float32).copy()      # [HD, S]
    sinT = np.sin(emb).T.astype(np.float32)
    sinTs = np.concatenate([-sinT[:HD // 2], sinT[HD // 2:]], 0).astype(
        np.float32).copy()
    return cosT, sinTs


def _mask_diag():
    # maskD[p, i] = 1.0 iff p <= i (upper-triangular incl. diagonal)
    p = np.arange(128)[:, None]
    i = np.arange(128)[None, :]
    return (p <= i).astype(BF16)


def _tile_w_fm(w, n_tiles, kt):
    # [K, n_tiles*128] -> [n_tiles, 128(p of K), kt, 128]
    K, N = w.shape
    assert K == kt * 128 and N == n_tiles * 128
    return np.ascontiguousarray(
        w.reshape(kt, 128, n_tiles, 128).transpose(2, 1, 0, 3)).astype(BF16)


def _tile_w_tm(w, kt):
    # [K, N] -> [128(p of K), kt, N]
    K, N = w.shape
    assert K == kt * 128
    return np.ascontiguousarray(
        w.reshape(kt, 128, N).transpose(1, 0, 2)).astype(BF16)


def _tile_w_out(w):
    # [1024, D] -> [8(nb), 128(p of rows), 8(h), 512]
    return np.ascontiguousarray(
        w.reshape(8, 128, D // 512, 512).transpose(2, 1, 0, 3)).astype(BF16)


def _make_in_maps(hidden_states, w_q, w_k, w_v, w_o, w_lq, w_lk, w_lv, w_lo):
    cosT, sinTs = _rope_tables()
    maskD = _mask_diag()
    w2 = np.asarray(w_lq, dtype=np.float32) @ np.asarray(w_lk,
                                                         dtype=np.float32)
    in_maps = []
    for c in range(NCORES):
        b, g = divmod(c, TP)
        x = np.asarray(hidden_states[b], dtype=np.float32)       # [S, D]
        xt = np.ascontiguousarray(
            x.T.reshape(D_T, 128, S).transpose(1, 0, 2)).astype(BF16)
        qs = slice(g * HL * HD, (g + 1) * HL * HD)
        kvs = slice(g * KVL * HD, (g + 1) * KVL * HD)
        ls = slice(g * LHL * HD, (g + 1) * LHL * HD)
        wv_t = _tile_w_tm(np.asarray(w_v)[:, kvs], D_T)
        wlv_t = _tile_w_tm(np.asarray(w_lv)[:, ls], D_T)
        in_maps.append({
            "xt": xt,
            "wq": _tile_w_fm(np.asarray(w_q)[:, qs], HL, D_T),
            "wk": _tile_w_fm(np.asarray(w_k)[:, kvs], KVL, D_T),
            "wvc": np.ascontiguousarray(
                np.concatenate([wlv_t[:, :, 0:256], wv_t], axis=2)),
            "w2": _tile_w_fm(w2[:, ls], LHL, D_T),
            "wlvb": np.ascontiguousarray(wlv_t[:, :, 256:768]),
            "wlvc": np.ascontiguousarray(wlv_t[:, :, 768:1024]),
            "wo": _tile_w_out(np.asarray(w_o)[qs, :]),
            "wlo": _tile_w_out(np.asarray(w_lo)[ls, :]),
            "cosT": cosT,
            "sinTs": sinTs,
            "maskD": maskD,
        })
    return in_maps


def kernel(hidden_states, w_q, w_k, w_v, w_o, w_lq, w_lk, w_lv, w_lo):
    nc = _get_program()
    in_maps = _make_in_maps(hidden_states, w_q, w_k, w_v, w_o,
                            w_lq, w_lk, w_lv, w_lo)
    res = run_bass_kernel_spmd(nc, in_maps, list(range(NCORES))).results

    out = np.zeros((B, S, D), dtype=np.float32)
    for c in range(NCORES):
        b = c // TP
        out[b] += res[c]["out"]
    return out
